# revision 8
# baseline (speedup 1.0000x reference)
"""Trainium2 Bass kernel for nn_NormLearningEngine.

Data-parallel over 8 NeuronCores: batch 64 -> 8 batches per core.
Per core the dominant work is action = x.mean(axis=1) over the core's
x shard, cast host-side to fp8 e4m3 (16 MiB per core; ~6e-5 output
rel-err vs the 2e-2 gate). The t-reduction runs on the PE as fp8
DoubleRow matmuls (two t-blocks contracted per pass): lhsT is a
[128, 2, G] one-hot mask, so each batch's sum lands in its own row of
a per-group [G, 1024] PSUM accumulator.

Batches stream in groups (3, 3, 1, 1); each group's MLP tail runs
deferred by one group so the PE queue never blocks the stream, and
the final batch's x is split into small DMAs so only a short catch-up
plus one 1-wide tail remains after the last HBM byte.

Scalar-engine table discipline: every gelu is computed as
0.5*x*(1+erf(x/sqrt2)) and erf/sigmoid/square/copy all live in the
sigmoid_and_others table set, the softmax exp is a degree-4 vector
polynomial, so the only table switches (sqrt for the rms-norm, then
back) happen once inside the context phase, fully overlapped.

Weights are host-packed bf16, split into a context-critical pack
(loaded first so the context pipeline never waits on the big
action-side blocks) and the action-side pack.
"""

import sys

sys.path.insert(0, "/opt/trn_rl_repo")

import numpy as np
import ml_dtypes

import concourse.bacc as bacc
import concourse.tile as tile
from concourse import mybir
from concourse.bass_utils import run_bass_kernel_spmd

F32 = mybir.dt.float32
BF16 = mybir.dt.bfloat16
FP8 = mybir.dt.float8e4
BF16NP = ml_dtypes.bfloat16
FP8NP = ml_dtypes.float8_e4m3fn
AF = mybir.ActivationFunctionType
ALU = mybir.AluOpType
AX = mybir.AxisListType
DR = mybir.MatmulPerfMode.DoubleRow

D, H, K, CTXW, T = 1024, 256, 64, 16, 2048
B, NCORES = 64, 8
BPC = B // NCORES  # 8 batches per core
ALPHA = 0.1
EPS = 1e-6
RSQRT2 = 0.7071067811865476

GROUPS = [(0, 3), (3, 3), (6, 1), (7, 1)]  # (first batch, width)

# vpack column map ([128, VCOLS] fp32)
C_ONES = 0       # all ones (col)
C_EPS = 1        # eps everywhere
C_RMSW = 2       # 8 cols: rms_w as columns
C_CEB1 = 10      # 2 cols
C_CEB2 = 12      # 8 cols
C_NMB1 = 20      # 4 cols
C_NSB1 = 24      # 2 cols
C_SVB1 = 26      # 2 cols
C_NSB2 = 34      # 1 col (rows 0:64)
C_NMB2 = 35      # 1 col (row 0)
C_SVB2 = 36      # 1 col (row 0)
C_EYE64 = 48     # 64 cols (rows 0:64 = eye(64))
C_ONESROW = 112  # 128 cols of ones (used as a [1,128] row)
VCOLS = 240

# bpack column map ([128, BCOLS] bf16)
BC_ONES = 0      # 8 cols all ones
BC_EYE8 = 8      # 8 cols (rows 0:8 = eye(8))
BC_SVW2 = 16     # 2 cols: sv_w2 chunks
BC_NMW2 = 18     # 4 cols: nm_w2 chunks
BCOLS = 22

# wpackA: context-critical blocks, chunk layout "(c p) m -> p (c m)"
A_W1 = 0                    # ce_w1  [1024, 256] -> 8*256
A_W2 = A_W1 + 8 * 256       # ce_w2  [256, 1024] -> 2*1024
A_NS1 = A_W2 + 2 * 1024     # ns_w1  [1024, 256] -> 8*256
A_NS2 = A_NS1 + 8 * 256     # ns_w2  [256, 64]   -> 2*64
A_NMC = A_NS2 + 2 * 64      # nm_w1[:1024] (wc)  -> 8*512
A_NMP = A_NMC + 8 * 512     # nm_w1[2048:] (wp)  -> 8*512
A_PTT = A_NMP + 8 * 512     # protos.T [1024,64] -> 8*64
A_SVC = A_PTT + 8 * 64      # sv_w1[:1024]       -> 8*256
WACOLS = A_SVC + 8 * 256    # 17024

# wpackB: action-side blocks
B_NMA = 0                   # nm_w1[1024:2048] (wa) -> 8*512
B_SVA = B_NMA + 8 * 512     # sv_w1[1024:]          -> 8*256
WBCOLS = B_SVA + 8 * 256    # 6144

# out vector layout (per core, [32])
O_NP, O_WC, O_VIOL, O_SEV = 0, 8, 16, 24

# last-batch DMA split along s (sizes summing to 8)
LAST_SPLIT = [4, 2, 2]


def build_program():
    nc = bacc.Bacc()

    # x layout per core: [b, p, j, s, d] with t = p*16 + j*8 + s
    x_d = nc.dram_tensor("x", [BPC, 128, 2, 8, D], FP8, kind="ExternalInput")
    cb_d = nc.dram_tensor("cb", [CTXW, D], F32, kind="ExternalInput")
    mp_d = nc.dram_tensor("mpack", [128, 2, 64], FP8, kind="ExternalInput")
    vp_d = nc.dram_tensor("vpack", [128, VCOLS], F32, kind="ExternalInput")
    bp_d = nc.dram_tensor("bpack", [128, BCOLS], BF16, kind="ExternalInput")
    wa_d = nc.dram_tensor("wpacka", [128, WACOLS], BF16, kind="ExternalInput")
    wb_d = nc.dram_tensor("wpackb", [128, WBCOLS], BF16, kind="ExternalInput")
    out_d = nc.dram_tensor("out", [32], F32, kind="ExternalOutput")

    mm = nc.tensor.matmul

    with tile.TileContext(nc) as tc:
        with (
            tc.tile_pool(name="const", bufs=1) as cp,
            tc.tile_pool(name="xin", bufs=6) as xp,
            tc.tile_pool(name="work", bufs=2) as wk,
            tc.tile_pool(name="ps_act", bufs=2, space="PSUM") as pa,
            tc.tile_pool(name="ps_t", bufs=2, space="PSUM") as pt,
        ):
            # ---- first x tile leads everything in the sync FIFO ----
            xt0 = xp.tile([128, 2, 8, D], FP8, tag="xt")
            nc.sync.dma_start(out=xt0[:], in_=x_d[0])

            # ---- constant / weight loads ----
            wpa = cp.tile([128, WACOLS], BF16)
            nc.sync.dma_start(out=wpa[:], in_=wa_d[:])
            vp = cp.tile([128, VCOLS], F32)
            nc.sync.dma_start(out=vp[:], in_=vp_d[:])
            bp = cp.tile([128, BCOLS], BF16)
            nc.sync.dma_start(out=bp[:], in_=bp_d[:])
            cb = cp.tile([CTXW, D], F32)
            nc.sync.dma_start(out=cb[:], in_=cb_d[:])
            mp = cp.tile([128, 2, 64], FP8)
            nc.sync.dma_start(out=mp[:], in_=mp_d[:])
            wpb = cp.tile([128, WBCOLS], BF16)
            nc.sync.dma_start(out=wpb[:], in_=wb_d[:])

            ones_row = vp[0:1, C_ONESROW : C_ONESROW + 128]  # [1, 128]
            eye64 = vp[0:64, C_EYE64 : C_EYE64 + 64]         # [64, 64]
            eye8b = bp[0:8, BC_EYE8 : BC_EYE8 + 8]           # [8, 8] bf16

            def wsa(base, chunk, m, cols, cw):
                c0 = base + chunk * cw + m * 128
                return wpa[:, c0 : c0 + cols]

            def wsb(base, chunk, m, cols, cw):
                c0 = base + chunk * cw + m * 128
                return wpb[:, c0 : c0 + cols]

            vadd = nc.vector.tensor_scalar_add
            vts = nc.vector.tensor_scalar
            vtt = nc.vector.tensor_tensor

            def gelu_erf(out_ap, pre_ap, e_ap):
                """out = 0.5 * pre * (1 + erf(pre/sqrt2)); erf stays in the
                sigmoid table set so no table switch."""
                nc.scalar.activation(out=e_ap, in_=pre_ap, func=AF.Erf,
                                     scale=RSQRT2)
                vts(out=e_ap, in0=e_ap, scalar1=0.5, scalar2=0.5,
                    op0=ALU.mult, op1=ALU.add)
                vtt(out=out_ap, in0=pre_ap, in1=e_ap, op=ALU.mult)

            # pin the sigmoid/erf table set before anything else on Scalar
            gwarm = cp.tile([1, 1], F32)

            # =========== phase X0: ctx column sums (only needs cb) ===========
            ctx_ps = pt.tile([128, 8], F32, tag="t")
            for c in range(8):
                mm(out=ctx_ps[:, c : c + 1], lhsT=cb[:, c * 128 : (c + 1) * 128],
                   rhs=vp[0:CTXW, C_ONES : C_ONES + 1], start=True, stop=True)
            nc.scalar.activation(out=gwarm[:], in_=vp[0:1, C_ONES : C_ONES + 1],
                                 func=AF.Erf)
            ctxTb = cp.tile([128, 8], BF16)
            nc.scalar.mul(out=ctxTb[:], in_=ctx_ps[:], mul=1.0 / CTXW)

            # =========== stream scaffolding ===========
            act_tiles = {}

            def group_x(g0, G):
                act = pa.tile([3, D], F32, tag="act")
                act_tiles[g0] = act
                for b in range(G):
                    batch = g0 + b
                    blk = b if G == 3 else 3
                    msk = mp[:, :, 16 * blk : 16 * blk + G]
                    first = b == 0
                    last = b == G - 1
                    if batch == 0:
                        tiles = [(xt0, 8)]
                    elif batch == BPC - 1:
                        tiles = []
                        s0 = 0
                        for ns in LAST_SPLIT:
                            xt = xp.tile([128, 2, ns, D], FP8, tag=f"xl{ns}",
                                         bufs=2)
                            nc.sync.dma_start(out=xt[:],
                                              in_=x_d[batch, :, :, s0 : s0 + ns])
                            tiles.append((xt, ns))
                            s0 += ns
                    else:
                        xt = xp.tile([128, 2, 8, D], FP8, tag="xt")
                        nc.sync.dma_start(out=xt[:], in_=x_d[batch])
                        tiles = [(xt, 8)]
                    si = 0
                    stot = sum(ns for _, ns in tiles)
                    for xt, ns in tiles:
                        for s in range(ns):
                            for ch in range(2):
                                mm(out=act[0:G, ch * 512 : (ch + 1) * 512],
                                   lhsT=msk,
                                   rhs=xt[:, :, s, ch * 512 : (ch + 1) * 512],
                                   start=(first and si == 0),
                                   stop=(last and si == stot - 1),
                                   perf_mode=DR)
                            si += 1

            # stream groups 0 and 1 ahead of the ctx pipeline (program order
            # on the PE queue; keeps the DMA stream free of stalls)
            group_x(*GROUPS[0])
            group_x(*GROUPS[1])

            # =========== T0: context pipeline (overlaps x streaming) ===========
            # ce layer 1: h1 = gelu(ce_w1.T @ ctx + ce_b1)  -> [128, 2] bf16
            h1_ps = pt.tile([128, 2], F32, tag="t")
            for m in range(2):
                for c in range(8):
                    mm(out=h1_ps[:, m : m + 1],
                       lhsT=wsa(A_W1, c, m, 128, 256),
                       rhs=ctxTb[:, c : c + 1], start=(c == 0), stop=(c == 7))
            h1pre = cp.tile([128, 2], F32)
            for m in range(2):
                vadd(out=h1pre[:, m : m + 1], in0=h1_ps[:, m : m + 1],
                     scalar1=vp[:, C_CEB1 + m : C_CEB1 + m + 1])
            h1e = cp.tile([128, 2], F32)
            h1b = cp.tile([128, 2], BF16)
            gelu_erf(h1b[:], h1pre[:], h1e[:])

            # ce layer 2: ctx_e = ce_w2.T @ h1 + ce_b2  -> [128, 8] f32
            ce_ps = pt.tile([128, 8], F32, tag="t")
            for m in range(8):
                for c in range(2):
                    mm(out=ce_ps[:, m : m + 1],
                       lhsT=wsa(A_W2, c, m, 128, 1024),
                       rhs=h1b[:, c : c + 1], start=(c == 0), stop=(c == 1))
            ctx_e = cp.tile([128, 8], F32)
            vtt(out=ctx_e[:], in0=ce_ps[:], in1=vp[:, C_CEB2 : C_CEB2 + 8],
                op=ALU.add)

            # rms norm: rstd = 1/sqrt(mean(ctx_e^2) + eps)
            sq = cp.tile([128, 8], F32)
            sqsum = cp.tile([128, 1], F32)
            nc.scalar.activation(out=sq[:], in_=ctx_e[:], func=AF.Square,
                                 accum_out=sqsum[:])
            ms_ps = pt.tile([1, 1], F32, tag="t")
            mm(out=ms_ps[:], lhsT=sqsum[:], rhs=vp[:, C_ONES : C_ONES + 1],
               start=True, stop=True)
            xms = cp.tile([1, 1], F32)
            vts(out=xms[:], in0=ms_ps[:], scalar1=1.0 / D, scalar2=EPS,
                op0=ALU.mult, op1=ALU.add)
            sd = cp.tile([1, 1], F32)
            nc.scalar.activation(out=sd[:], in_=ms_ps[:], func=AF.Sqrt,
                                 bias=vp[0:1, C_EPS : C_EPS + 1], scale=1.0 / D)
            r = cp.tile([1, 1], F32)
            nc.vector.reciprocal(out=r[:], in_=sd[:])
            tmp1 = cp.tile([1, 1], F32)
            for _ in range(2):  # r <- r*(1.5 - 0.5*x*r^2)
                vtt(out=tmp1[:], in0=r[:], in1=r[:], op=ALU.mult)
                vtt(out=tmp1[:], in0=tmp1[:], in1=xms[:], op=ALU.mult)
                vts(out=tmp1[:], in0=tmp1[:], scalar1=-0.5, scalar2=1.5,
                    op0=ALU.mult, op1=ALU.add)
                vtt(out=r[:], in0=r[:], in1=tmp1[:], op=ALU.mult)
            rb_ps = pt.tile([128, 1], F32, tag="t")
            mm(out=rb_ps[:], lhsT=ones_row, rhs=r[:], start=True, stop=True)
            rb = cp.tile([128, 1], F32)
            nc.vector.tensor_copy(out=rb[:], in_=rb_ps[:])
            ctx_enc = cp.tile([128, 8], F32)
            vtt(out=ctx_enc[:], in0=ctx_e[:], in1=vp[:, C_RMSW : C_RMSW + 8],
                op=ALU.mult)
            nc.vector.tensor_scalar_mul(out=ctx_enc[:], in0=ctx_enc[:],
                                        scalar1=rb[:])
            ctxEb = cp.tile([128, 8], BF16)
            nc.vector.tensor_copy(out=ctxEb[:], in_=ctx_enc[:])

            # norm selector: s1 = gelu(ns_w1.T @ ctx_enc + ns_b1) -> [128, 2]
            s1_ps = pt.tile([128, 2], F32, tag="t")
            for m in range(2):
                for c in range(8):
                    mm(out=s1_ps[:, m : m + 1],
                       lhsT=wsa(A_NS1, c, m, 128, 256),
                       rhs=ctxEb[:, c : c + 1], start=(c == 0), stop=(c == 7))
            s1pre = cp.tile([128, 2], F32)
            for m in range(2):
                vadd(out=s1pre[:, m : m + 1], in0=s1_ps[:, m : m + 1],
                     scalar1=vp[:, C_NSB1 + m : C_NSB1 + m + 1])
            s1e = cp.tile([128, 2], F32)
            s1b = cp.tile([128, 2], BF16)
            gelu_erf(s1b[:], s1pre[:], s1e[:])

            # logits = ns_w2.T @ s1 + ns_b2 -> [64, 1] column
            lg_ps = pt.tile([64, 1], F32, tag="t")
            for c in range(2):
                mm(out=lg_ps[:], lhsT=wsa(A_NS2, c, 0, 64, 64),
                   rhs=s1b[:, c : c + 1], start=(c == 0), stop=(c == 1))
            lgc = cp.tile([64, 1], F32)
            vadd(out=lgc[:], in0=lg_ps[:], scalar1=vp[0:64, C_NSB2 : C_NSB2 + 1])
            # transpose to row via eye64
            lr_ps = pt.tile([1, 64], F32, tag="t")
            mm(out=lr_ps[:], lhsT=lgc[:], rhs=eye64, start=True, stop=True)
            lrow = cp.tile([1, 64], F32)
            nc.vector.tensor_copy(out=lrow[:], in_=lr_ps[:])
            # softmax via vector-engine exp polynomial (logits are tiny)
            mx = cp.tile([1, 1], F32)
            nc.vector.tensor_reduce(out=mx[:], in_=lrow[:], axis=AX.X, op=ALU.max)
            nmx = cp.tile([1, 1], F32)
            nc.vector.tensor_scalar_mul(out=nmx[:], in0=mx[:], scalar1=-1.0)
            z = cp.tile([1, 64], F32)
            vadd(out=z[:], in0=lrow[:], scalar1=nmx[:])
            ex = cp.tile([1, 64], F32)
            # exp(z) ~= (((z/24 + 1/6)z + 1/2)z + 1)z + 1 for z in [-r, 0]
            vts(out=ex[:], in0=z[:], scalar1=1.0 / 24, scalar2=1.0 / 6,
                op0=ALU.mult, op1=ALU.add)
            vtt(out=ex[:], in0=ex[:], in1=z[:], op=ALU.mult)
            vadd(out=ex[:], in0=ex[:], scalar1=0.5)
            vtt(out=ex[:], in0=ex[:], in1=z[:], op=ALU.mult)
            vadd(out=ex[:], in0=ex[:], scalar1=1.0)
            vtt(out=ex[:], in0=ex[:], in1=z[:], op=ALU.mult)
            vadd(out=ex[:], in0=ex[:], scalar1=1.0)
            exsum = cp.tile([1, 1], F32)
            nc.vector.tensor_reduce(out=exsum[:], in_=ex[:], axis=AX.X,
                                    op=ALU.add)
            rexs = cp.tile([1, 1], F32)
            nc.vector.reciprocal(out=rexs[:], in_=exsum[:])
            nw = cp.tile([1, 64], F32)
            nc.vector.tensor_scalar_mul(out=nw[:], in0=ex[:], scalar1=rexs[:])
            nw3 = cp.tile([1, 3 * K], F32)
            for b in range(3):
                nc.vector.tensor_copy(out=nw3[:, b * K : (b + 1) * K], in_=nw[:])

            # nm ctx part: u = wc.T @ ctx_enc + nm_b1 -> [128, 4]
            u_ps = pt.tile([128, 4], F32, tag="t")
            for hc in range(4):
                for c in range(8):
                    mm(out=u_ps[:, hc : hc + 1],
                       lhsT=wsa(A_NMC, c, hc, 128, 512),
                       rhs=ctxEb[:, c : c + 1], start=(c == 0), stop=(c == 7))
            u = cp.tile([128, 4], F32)
            vtt(out=u[:], in0=u_ps[:], in1=vp[:, C_NMB1 : C_NMB1 + 4], op=ALU.add)

            # nm proto part: PT = wp_w.T @ protosT -> [128, 4*64]
            pt_ps = pt.tile([128, 4 * K], F32, tag="t")
            for hc in range(4):
                for c in range(8):
                    mm(out=pt_ps[:, hc * K : (hc + 1) * K],
                       lhsT=wsa(A_NMP, c, hc, 128, 512),
                       rhs=wpa[:, A_PTT + c * K : A_PTT + (c + 1) * K],
                       start=(c == 0), stop=(c == 7))
            PTs = cp.tile([128, 4 * K], F32)
            nc.vector.tensor_copy(out=PTs[:], in_=pt_ps[:])

            # severity ctx part: svu = sv_w1[:D].T @ ctx_enc + sv_b1 -> [128, 2]
            svu_ps = pt.tile([128, 2], F32, tag="t")
            for m in range(2):
                for c in range(8):
                    mm(out=svu_ps[:, m : m + 1],
                       lhsT=wsa(A_SVC, c, m, 128, 256),
                       rhs=ctxEb[:, c : c + 1], start=(c == 0), stop=(c == 7))
            svu = cp.tile([128, 2], F32)
            vtt(out=svu[:], in0=svu_ps[:], in1=vp[:, C_SVB1 : C_SVB1 + 2],
                op=ALU.add)

            out_sb = cp.tile([1, 32], F32)

            # =========== per-group MLP tail ===========
            def group_tail(g0, G):
                act = act_tiles[g0]
                # action rows (scaled 1/T) -> bf16
                actR = wk.tile([3, D], BF16, tag="actR")
                nc.vector.tensor_scalar_mul(out=actR[0:G, :], in0=act[0:G, :],
                                            scalar1=1.0 / T)
                # transpose to columns: trp[:, c, :] = actR[:, cchunk].T
                trp = pt.tile([128, 8, 3], F32, tag="grp")
                for c in range(8):
                    mm(out=trp[:, c, 0:G],
                       lhsT=actR[0:G, c * 128 : (c + 1) * 128],
                       rhs=eye8b[0:G, 0:G], start=True, stop=True)
                aT = wk.tile([128, 8, 3], BF16, tag="aT")
                nc.vector.tensor_copy(out=aT[:, :, 0:G], in_=trp[:, :, 0:G])

                # nm action part: base = wa.T @ actionT (+u) -> [128, 4, G]
                base_ps = pt.tile([128, 4, 3], F32, tag="grp")
                for hc in range(4):
                    for c in range(8):
                        mm(out=base_ps[:, hc, 0:G],
                           lhsT=wsb(B_NMA, c, hc, 128, 512),
                           rhs=aT[:, c, 0:G], start=(c == 0), stop=(c == 7))
                ub = wk.tile([128, 4, 3], F32, tag="ub")
                for hc in range(4):
                    vadd(out=ub[:, hc, 0:G], in0=base_ps[:, hc, 0:G],
                         scalar1=u[:, hc : hc + 1])

                # conformance gelu via erf: pre = PTs + ub[:, :, b]
                pre = wk.tile([128, 3, 4, K], F32, tag="pre")
                for b in range(G):
                    for hc in range(4):
                        vadd(out=pre[:, b, hc, :],
                             in0=PTs[:, hc * K : (hc + 1) * K],
                             scalar1=ub[:, hc, b : b + 1])
                ech = wk.tile([128, 3, 4, K], F32, tag="ech")
                gch = wk.tile([128, 3, 4, K], BF16, tag="g")
                gelu_erf(gch[:, 0:G], pre[:, 0:G], ech[:, 0:G])

                conf_ps = pt.tile([1, 3 * K], F32, tag="grp")
                for b in range(G):
                    for hc in range(4):
                        mm(out=conf_ps[:, b * K : (b + 1) * K],
                           lhsT=bp[:, BC_NMW2 + hc : BC_NMW2 + hc + 1],
                           rhs=gch[:, b, hc, :],
                           start=(hc == 0), stop=(hc == 3))
                confr = wk.tile([1, 3 * K], F32, tag="confr")
                nc.scalar.activation(out=confr[0:1, 0 : G * K],
                                     in_=conf_ps[0:1, 0 : G * K],
                                     func=AF.Sigmoid,
                                     bias=vp[0:1, C_NMB2 : C_NMB2 + 1])
                prod = wk.tile([1, 3 * K], F32, tag="prod")
                vtt(out=prod[0:1, 0 : G * K], in0=confr[0:1, 0 : G * K],
                    in1=nw3[0:1, 0 : G * K], op=ALU.mult)
                nc.vector.tensor_reduce(
                    out=out_sb[0:1, O_WC + g0 : O_WC + g0 + G],
                    in_=prod[0:1, 0 : G * K].rearrange("p (b k) -> p b k", b=G),
                    axis=AX.X, op=ALU.add)

                # severity head
                sv_ps = pt.tile([128, 2, 3], F32, tag="grp")
                for m in range(2):
                    for c in range(8):
                        mm(out=sv_ps[:, m, 0:G],
                           lhsT=wsb(B_SVA, c, m, 128, 256),
                           rhs=aT[:, c, 0:G], start=(c == 0), stop=(c == 7))
                spre = wk.tile([128, 2, 3], F32, tag="spre")
                for m in range(2):
                    vadd(out=spre[:, m, 0:G], in0=sv_ps[:, m, 0:G],
                         scalar1=svu[:, m : m + 1])
                se = wk.tile([128, 2, 3], F32, tag="se")
                svg = wk.tile([128, 2, 3], BF16, tag="svg")
                gelu_erf(svg[:, :, 0:G], spre[:, :, 0:G], se[:, :, 0:G])
                sev_ps = pt.tile([1, 3], F32, tag="grp")
                for m in range(2):
                    mm(out=sev_ps[0:1, 0:G],
                       lhsT=bp[:, BC_SVW2 + m : BC_SVW2 + m + 1],
                       rhs=svg[:, m, 0:G], start=(m == 0), stop=(m == 1))
                nc.scalar.activation(out=out_sb[0:1, O_SEV + g0 : O_SEV + g0 + G],
                                     in_=sev_ps[0:1, 0:G],
                                     func=AF.Sigmoid,
                                     bias=vp[0:1, C_SVB2 : C_SVB2 + 1])

            # deferred-by-one pipeline: tails trail the stream by one group
            group_tail(*GROUPS[0])
            group_x(*GROUPS[2])
            group_tail(*GROUPS[1])
            group_x(*GROUPS[3])
            group_tail(*GROUPS[2])
            group_tail(*GROUPS[3])

            # =========== final combine ===========
            vts(out=out_sb[0:1, O_VIOL : O_VIOL + 8],
                in0=out_sb[0:1, O_WC : O_WC + 8],
                scalar1=-1.0, scalar2=1.0, op0=ALU.mult, op1=ALU.add)
            vtt(out=out_sb[0:1, O_NP : O_NP + 8],
                in0=out_sb[0:1, O_VIOL : O_VIOL + 8],
                in1=out_sb[0:1, O_SEV : O_SEV + 8], op=ALU.mult)
            nc.vector.tensor_scalar_mul(out=out_sb[0:1, O_NP : O_NP + 8],
                                        in0=out_sb[0:1, O_NP : O_NP + 8],
                                        scalar1=ALPHA)

            nc.sync.dma_start(out=out_d[:].rearrange("(p n) -> p n", p=1),
                              in_=out_sb[0:1, :])

    nc.finalize()
    return nc


def _build_vpack(inp):
    vp = np.zeros((128, VCOLS), np.float32)

    def cols(v, c0):
        v = np.asarray(v, np.float32).reshape(-1)
        ncols = (len(v) + 127) // 128
        for c in range(ncols):
            seg = v[c * 128 : (c + 1) * 128]
            vp[: len(seg), c0 + c] = seg

    vp[:, C_ONES] = 1.0
    vp[:, C_EPS] = EPS
    cols(inp["rms_w"], C_RMSW)
    cols(inp["ce_b1"], C_CEB1)
    cols(inp["ce_b2"], C_CEB2)
    cols(inp["nm_b1"], C_NMB1)
    cols(inp["ns_b1"], C_NSB1)
    cols(inp["sv_b1"], C_SVB1)
    cols(inp["ns_b2"], C_NSB2)
    cols(inp["nm_b2"], C_NMB2)
    cols(inp["sv_b2"], C_SVB2)
    vp[0:64, C_EYE64 : C_EYE64 + 64] = np.eye(64, dtype=np.float32)
    vp[0, C_ONESROW : C_ONESROW + 128] = 1.0
    return vp


def _build_bpack(inp):
    bp = np.zeros((128, BCOLS), np.float32)
    bp[:, BC_ONES : BC_ONES + 8] = 1.0
    bp[0:8, BC_EYE8 : BC_EYE8 + 8] = np.eye(8, dtype=np.float32)
    sv_w2 = np.asarray(inp["sv_w2"], np.float32).reshape(-1)
    for c in range(2):
        bp[:, BC_SVW2 + c] = sv_w2[c * 128 : (c + 1) * 128]
    nm_w2 = np.asarray(inp["nm_w2"], np.float32).reshape(-1)
    for c in range(4):
        bp[:, BC_NMW2 + c] = nm_w2[c * 128 : (c + 1) * 128]
    return bp.astype(BF16NP)


def _pack_into(dst, w, c0):
    w = np.asarray(w, np.float32)
    ck, m = w.shape[0] // 128, w.shape[1]
    for c in range(ck):
        dst[:, c0 + c * m : c0 + (c + 1) * m] = w[c * 128 : (c + 1) * 128].astype(
            dst.dtype
        )


def _build_wpacks(inp):
    nm = np.asarray(inp["nm_w1"], np.float32)
    sv = np.asarray(inp["sv_w1"], np.float32)
    wa = np.zeros((128, WACOLS), BF16NP)
    _pack_into(wa, inp["ce_w1"], A_W1)
    _pack_into(wa, inp["ce_w2"], A_W2)
    _pack_into(wa, inp["ns_w1"], A_NS1)
    _pack_into(wa, inp["ns_w2"], A_NS2)
    _pack_into(wa, nm[:1024], A_NMC)
    _pack_into(wa, nm[2048:], A_NMP)
    _pack_into(wa, np.asarray(inp["norm_prototypes"], np.float32).T, A_PTT)
    _pack_into(wa, sv[:1024], A_SVC)
    wb = np.zeros((128, WBCOLS), BF16NP)
    _pack_into(wb, nm[1024:2048], B_NMA)
    _pack_into(wb, sv[1024:], B_SVA)
    return wa, wb


_CACHE = {}


def _in_maps(inputs):
    npin = {k: np.asarray(v) for k, v in inputs.items()}
    # x -> fp8 e4m3, laid out [b, p, j, s, d] with t = p*16 + j*8 + s
    x = np.asarray(npin["x"], np.float32).astype(FP8NP)
    x = np.ascontiguousarray(x.reshape(B, 128, 2, 8, D))
    mpack = np.zeros((128, 2, 64), FP8NP)
    for i, b in enumerate((0, 1, 2, 0)):  # blocks: G=3 b=0..2, G=1 b=0
        mpack[:, :, 16 * i + b] = 1.0
    wa, wb = _build_wpacks(npin)
    shared = {
        "cb": np.ascontiguousarray(np.asarray(npin["context_buffer"], np.float32)
                                   .reshape(CTXW, D)),
        "vpack": _build_vpack(npin),
        "bpack": _build_bpack(npin),
        "wpacka": wa,
        "wpackb": wb,
        "mpack": mpack,
    }
    return [dict(shared, x=np.ascontiguousarray(x[c * BPC : (c + 1) * BPC]))
            for c in range(NCORES)]


def run(inputs, trace=False, tmpdir=None):
    if "nc" not in _CACHE:
        _CACHE["nc"] = build_program()
    res = run_bass_kernel_spmd(_CACHE["nc"], _in_maps(inputs),
                               list(range(NCORES)), trace=trace, tmpdir=tmpdir)
    npen = np.empty(B, np.float32)
    wc = np.empty(B, np.float32)
    viol = np.empty(B, np.float32)
    sev = np.empty(B, np.float32)
    for c in range(NCORES):
        o = res.results[c]["out"]
        npen[c * BPC : (c + 1) * BPC] = o[O_NP : O_NP + 8]
        wc[c * BPC : (c + 1) * BPC] = o[O_WC : O_WC + 8]
        viol[c * BPC : (c + 1) * BPC] = o[O_VIOL : O_VIOL + 8]
        sev[c * BPC : (c + 1) * BPC] = o[O_SEV : O_SEV + 8]
    return (npen, wc, viol, sev), res


def kernel(**inputs):
    outs, _ = run(inputs, trace=False)
    return outs
